# revision 28
# baseline (speedup 1.0000x reference)
"""Circular-relative-bias multi-head attention on 8 Trainium2 NeuronCores.

Sharding (Megatron MHA): 16 heads -> 2 heads per core. Each core computes
q/k/v projections for its 128 channels (2 heads x 64), full attention for
its heads over both batches, and a row-sharded output projection producing
a full-shape partial; the host sums the 8 partials and adds bo.

Layout strategy: the host pre-packs every input into the exact SBUF layout
the kernel wants, so all DMAs are linear:
  - xt      [128, 16, 8, 256]   x transposed, tiled per token-block so each
                                block is one contiguous run per partition
  - wq/wk/wv[128, 8, 128]       [k-part, d-tile, ch]         bf16 (wq,bq pre-scaled 1/8)
  - wo      [128, 1024]         [ch, d]                      bf16
  - bq/bk/bv[128, 1]            per-channel bias             f32
  - ebias   [128, 2, 28, 512]   exp(rel bias) tiles, by      bf16
                                [i, head, diag-class, j]

Attention works on transposed scores P^T [sk, sq] so softmax sums come free
from the attn@V matmul via a ones-column in the stationary operand:
  head0 lhsT = v_store[:, t, 0:65]  = [v0 | ones]      -> accA rows 0-63 data, 64 sums
  head1 lhsT = v_store[:, t, 32:160] = [.. | ones | v1] -> accB row 32 sums, rows 64-127 data

exp(s + b) = exp(s) * exp(b): the circular bias enters as a precomputed
elementwise bf16 multiplier; [128, 512] score tiles along the same
(512*qb - 128*kt) diagonal share one of 28 classes per head.

Engine schedule: ONE flat software pipeline over all 128 (b, qb, kt)
iterations -- QK is issued two slots ahead and PV lags one slot behind, both
crossing qb boundaries, so the per-slot steady state (exp on scalar ~1.0us,
bias-mul on vector/gpsimd, 2xQK + 2xPV on tensor) never drains at a qb edge.
Per-qb epilogue work (reciprocal straight from PSUM, gpsimd partition
broadcast, normalize muls, 8 output-projection matmuls + psum->sbuf casts +
HBM writeback) is sliced into per-slot deferred closures spread across the
following qb's 16 slots. PSUM: scores 2x2 banks, PV accum 3x1, outproj 1x1.

No max-subtraction in softmax: scores ~ N(0,1) + 0.02-bias, |s| < ~7 over
4M samples, exp stays well inside f32 range.
"""

import math
from collections import defaultdict

import numpy as np
import ml_dtypes

B = 2
S = 2048
D = 1024
H = 16
HD = 64
PERIOD = 4096
NCORES = 8
CH = 128          # channels per core = 2 heads * 64
TOK = B * S       # 4096
DT = D // 128     # 8 k-tiles for the d contraction
TB = 256          # token block for projections
NTB = TOK // TB   # 16
SQ = 512          # sq block in attention
NQB = S // SQ     # 4 per batch
SK = 128          # sk tile
NKT = S // SK     # 16 per batch
NCLS = NQB * 4 + NKT - 4  # 28 diagonal classes: 4*qb - kt in [-15, 12]
NIT = B * NQB * NKT       # 128 flat attention iterations

_CACHE = {}


def _build_nc():
    import contextlib

    import concourse.tile as tile
    from concourse import bacc, mybir
    from concourse.masks import make_identity

    f32 = mybir.dt.float32
    bf16 = mybir.dt.bfloat16

    nc = bacc.Bacc("TRN2")
    xt = nc.dram_tensor("xt", [128, NTB, DT, TB], bf16, kind="ExternalInput")
    wq = nc.dram_tensor("wq", [128, DT, CH], bf16, kind="ExternalInput")
    wk = nc.dram_tensor("wk", [128, DT, CH], bf16, kind="ExternalInput")
    wv = nc.dram_tensor("wv", [128, DT, CH], bf16, kind="ExternalInput")
    wo = nc.dram_tensor("wo", [CH, D], bf16, kind="ExternalInput")
    bq = nc.dram_tensor("bq", [CH, 1], f32, kind="ExternalInput")
    bk = nc.dram_tensor("bk", [CH, 1], f32, kind="ExternalInput")
    bv = nc.dram_tensor("bv", [CH, 1], f32, kind="ExternalInput")
    ebias = nc.dram_tensor("ebias", [128, 2, NCLS, SQ], bf16, kind="ExternalInput")
    o_part = nc.dram_tensor("o_part", [TOK, D], bf16, kind="ExternalOutput")

    with tile.TileContext(nc) as tc, contextlib.ExitStack() as ctx:
        singles = ctx.enter_context(tc.tile_pool(name="singles", bufs=1))
        xt_pool = ctx.enter_context(tc.tile_pool(name="xt", bufs=6))
        vt_pool = ctx.enter_context(tc.tile_pool(name="vt", bufs=2))
        e_pool = ctx.enter_context(tc.tile_pool(name="ep", bufs=3))
        p_pool = ctx.enter_context(tc.tile_pool(name="pp", bufs=4))
        nrm_pool = ctx.enter_context(tc.tile_pool(name="nrm", bufs=6))
        ao_pool = ctx.enter_context(tc.tile_pool(name="ao", bufs=2))
        ob_pool = ctx.enter_context(tc.tile_pool(name="ob", bufs=3))
        # PSUM budget (8 banks): scores 2x2 + PV accum 3x1 + outproj 1x1.
        sc_ps = ctx.enter_context(tc.tile_pool(name="scps", bufs=2, space="PSUM"))
        acc_ps = ctx.enter_context(tc.tile_pool(name="accps", bufs=3, space="PSUM"))
        op_ps = ctx.enter_context(tc.tile_pool(name="opps", bufs=1, space="PSUM"))

        ident = singles.tile([128, 128], bf16)
        make_identity(nc, ident)

        # q^T / k^T stores [ch, tok]; v_store [tok-part, tok-tile, 160]
        qT = singles.tile([CH, TOK], bf16, tag="qT")
        kT = singles.tile([CH, TOK], bf16, tag="kT")
        # v_store cols: [v0: 0..63 | ones: 64 | zeros: 65..95 | v1: 96..159]
        v_store = singles.tile([128, TOK // 128, 160], bf16, tag="vst")
        nc.vector.memset(v_store[:, :, 64:65], 1.0)
        nc.vector.memset(v_store[:, :, 65:96], 0.0)

        # ---- input DMAs: first xt tiles win early HBM bandwidth ----
        w_sb = {}
        b_sb = {}
        for name, w_h, b_h in (("q", wq, bq), ("k", wk, bk), ("v", wv, bv)):
            w_sb[name] = singles.tile([128, DT, CH], bf16, tag=f"w{name}", name=f"w{name}_sb")
            nc.sync.dma_start(out=w_sb[name], in_=w_h[:, :, :])
            b_sb[name] = singles.tile([CH, 1], f32, tag=f"b{name}", name=f"b{name}_sb")
            nc.sync.dma_start(out=b_sb[name], in_=b_h[:, :])

        # attention-phase weights, interleaved into the projection loop so
        # they stream behind the xt tiles without delaying them
        eb_sb = singles.tile([128, 2, NCLS, SQ], bf16, tag="eb")
        wo_sb = singles.tile([CH, D], bf16, tag="wo")

        # ---- projections ----
        for tb in range(NTB):
            ts = tb * TB
            xt_sb = xt_pool.tile([128, DT, TB], bf16, tag="xt")
            nc.sync.dma_start(out=xt_sb, in_=xt[:, tb])
            for name, store in (("q", qT), ("k", kT), ("v", None)):
                ps = sc_ps.tile([CH, TB], f32, tag="mm")
                for dt in range(DT):
                    nc.tensor.matmul(
                        ps,
                        w_sb[name][:, dt],
                        xt_sb[:, dt],
                        start=(dt == 0),
                        stop=(dt == DT - 1),
                    )
                if store is not None:
                    nc.scalar.add(store[:, ts : ts + TB], ps, b_sb[name])
                else:
                    vt_sb = vt_pool.tile([CH, TB], bf16, tag="vt")
                    nc.scalar.add(vt_sb, ps, b_sb["v"])
                    # transpose v^T -> v rows, split heads into v_store
                    for j in range(TB // 128):
                        t_idx = (ts + j * 128) // 128
                        vps = acc_ps.tile([128, 128], bf16, tag="acc")
                        nc.tensor.transpose(
                            vps, vt_sb[:, j * 128 : (j + 1) * 128], ident
                        )
                        nc.vector.tensor_copy(
                            v_store[:, t_idx, 0:64], vps[:, 0:64]
                        )
                        nc.vector.tensor_copy(
                            v_store[:, t_idx, 96:160], vps[:, 64:128]
                        )
            if tb == 5:
                nc.sync.dma_start(out=eb_sb[:, :, 0:8, :], in_=ebias[:, :, 0:8, :])
            elif tb == 10:
                nc.sync.dma_start(out=eb_sb[:, :, 8:16, :], in_=ebias[:, :, 8:16, :])
            elif tb == 13:
                nc.sync.dma_start(out=wo_sb, in_=wo[:, :])
            elif tb == 15:
                # classes 22-27 are first read in qb 2 (~35us into attention);
                # their DMA is deferred into the attention phase below
                nc.sync.dma_start(
                    out=eb_sb[:, :, 16:22, :], in_=ebias[:, :, 16:22, :]
                )

        # ---- attention: one flat software pipeline over 128 iterations ----
        def it_info(i):
            b, r = divmod(i, NQB * NKT)
            qb, kt = divmod(r, NKT)
            return b, qb, kt

        sc_tiles = {}
        acc = {}
        # pre_d runs at the top of a slot (amuls, psum->sbuf casts); post_d at
        # the bottom (outproj matmuls) so an outproj MM still waiting on the
        # normalize chain sits BEHIND that slot's QK/PV in the tensor FIFO
        # instead of head-of-line blocking them
        pre_d = defaultdict(list)
        post_d = defaultdict(list)

        def qk_flat(i):
            b, qb, kt = it_info(i)
            sc = sc_ps.tile([128, 2, SQ], f32, tag="mm", name=f"sc_{i}")
            sc_tiles[i] = sc
            base = b * S
            q0 = base + qb * SQ
            k0 = base + kt * SK
            for hh in (0, 1):
                nc.tensor.matmul(
                    sc[:, hh, :],
                    kT[hh * 64 : (hh + 1) * 64, k0 : k0 + SK],
                    qT[hh * 64 : (hh + 1) * 64, q0 : q0 + SQ],
                    start=True,
                    stop=True,
                )

        def emit_epilogue(slot, b, qb, accA, accB, last=False):
            """Normalize + outproj for finished qb, spread over next slots."""
            # reciprocal straight from the PSUM sum rows (head0: accA row 64,
            # head1: accB row 32), broadcast over the 64 head channels
            srow0 = nrm_pool.tile([1, SQ], f32, tag="srow")
            srow1 = nrm_pool.tile([1, SQ], f32, tag="srow")
            nc.vector.tensor_copy(srow0, accA[64:65])
            nc.vector.tensor_copy(srow1, accB[32:33])
            rr0 = nrm_pool.tile([1, SQ], f32, tag="rr")
            rr1 = nrm_pool.tile([1, SQ], f32, tag="rr")
            nc.vector.reciprocal_approx_fast(out=rr0, in_=srow0)
            nc.vector.reciprocal_approx_fast(out=rr1, in_=srow1)
            rb0 = nrm_pool.tile([64, SQ], f32, tag="rb")
            rb1 = nrm_pool.tile([64, SQ], f32, tag="rb")
            nc.gpsimd.partition_broadcast(rb0, rr0)
            nc.gpsimd.partition_broadcast(rb1, rr1)
            attnout = ao_pool.tile([128, SQ], bf16, tag="ao")

            def amuls():
                nc.vector.tensor_mul(attnout[0:64], accA[0:64], rb0)
                nc.vector.tensor_mul(attnout[64:128], accB[64:128], rb1)

            pre_d[slot + 1].append(amuls)

            base_q = b * S + qb * SQ
            if last:
                # drain tail: scores PSUM is free, so pipeline full-width
                # outproj tiles through sc_ps with casts on scalar+vector
                for ts in range(4):

                    def run2(ts=ts):
                        op = sc_ps.tile([128, 2, SQ], f32, tag="mm", name=f"opd_{ts}")
                        for half in range(2):
                            nc.tensor.matmul(
                                op[:, half, :],
                                attnout[:, ts * 128 : (ts + 1) * 128],
                                wo_sb[:, half * 512 : (half + 1) * 512],
                                start=True,
                                stop=True,
                            )

                        def cp(op=op, ts=ts):
                            ob = ob_pool.tile([128, 2, SQ], bf16, tag="ob2")
                            if ts % 2:
                                nc.scalar.copy(ob, op)
                            else:
                                nc.vector.tensor_copy(ob, op)
                            r0 = base_q + ts * 128
                            nc.sync.dma_start(out=o_part[r0 : r0 + 128, :], in_=ob)

                        pre_d[slot + 3 + ts].append(cp)

                    post_d[slot + 2 + ts].append(run2)
                return
            for k in range(8):
                ts, half = divmod(k, 2)

                def run_mm(ts=ts, half=half):
                    op = op_ps.tile([128, SQ], f32, tag="op")
                    nc.tensor.matmul(
                        op,
                        attnout[:, ts * 128 : (ts + 1) * 128],
                        wo_sb[:, half * 512 : (half + 1) * 512],
                        start=True,
                        stop=True,
                    )

                    def run_cp(op=op, ts=ts, half=half, k=k):
                        ob = ob_pool.tile([128, SQ], bf16, tag="ob")
                        # balance psum->sbuf casts: 2 of 8 ride the scalar
                        # queue (vector is the tightest engine in steady state)
                        if k in (1, 5):
                            nc.scalar.copy(ob, op)
                        else:
                            nc.vector.tensor_copy(ob, op)
                        r0 = base_q + ts * 128
                        nc.sync.dma_start(
                            out=o_part[r0 : r0 + 128, half * 512 : (half + 1) * 512],
                            in_=ob,
                        )

                    pre_d[slot + 5 + 2 * k].append(run_cp)

                post_d[slot + 4 + 2 * k].append(run_mm)

        def emit_pv(j):
            b, qb, kt = it_info(j)
            if kt == 0:
                acc[(b, qb)] = (
                    acc_ps.tile([128, SQ], f32, tag="acc", name=f"accA_{b}_{qb}"),
                    acc_ps.tile([128, SQ], f32, tag="acc", name=f"accB_{b}_{qb}"),
                )
            accA, accB = acc[(b, qb)]
            t_idx = (b * S + kt * SK) // 128
            p_sb = p_tiles.pop(j)
            for hh, a in ((0, accA), (1, accB)):
                lo, width = (0, 65) if hh == 0 else (32, 128)
                nc.tensor.matmul(
                    a[0:width, :],
                    v_store[:, t_idx, lo : lo + width],
                    p_sb[:, hh, :],
                    start=(kt == 0),
                    stop=(kt == NKT - 1),
                )
            return kt == NKT - 1

        # bias-muls on these kt go to gpsimd (~3x slower than DVE), so their
        # PV is pushed one extra slot to keep the tensor FIFO from stalling
        GP_KTS = set()  # gpsimd elementwise offload measured slower on hw
        p_tiles = {}
        pv_due = defaultdict(list)
        for i in range(NIT + 24):
            for fn in pre_d.pop(i, []):
                fn()
            if i == 0:
                qk_flat(0)
                qk_flat(1)
            if i == 2:
                nc.sync.dma_start(
                    out=eb_sb[:, :, 22:28, :], in_=ebias[:, :, 22:28, :]
                )
            if i < NIT:
                b, qb, kt = it_info(i)
                cls = 4 * qb - kt + (NKT - 1)
                e_sb = e_pool.tile([128, 2, SQ], bf16, tag="e")
                nc.scalar.activation(
                    out=e_sb, in_=sc_tiles.pop(i),
                    func=mybir.ActivationFunctionType.Exp,
                )
                p_sb = p_pool.tile([128, 2, SQ], bf16, tag="p")
                eng = nc.gpsimd if kt in GP_KTS else nc.vector
                eng.tensor_mul(p_sb, e_sb, eb_sb[:, :, cls, :])
                p_tiles[i] = p_sb
                pv_due[i + (2 if kt in GP_KTS else 1)].append(i)
                if i + 2 < NIT:
                    qk_flat(i + 2)
            for j in sorted(pv_due.pop(i, [])):
                if emit_pv(j):
                    jb, jqb, _ = it_info(j)
                    a, bacc_t = acc[(jb, jqb)]
                    emit_epilogue(i, jb, jqb, a, bacc_t, last=(j == NIT - 1))
            for fn in post_d.pop(i, []):
                fn()
        assert not pre_d and not post_d and not p_tiles and not sc_tiles
    nc.compile()
    return nc


def _prep_inputs(x, wq, bq, wk, bk, wv, bv, wo, bo, rel_bias):
    """Host-side pack into per-core in_maps (all linear-DMA layouts)."""
    x = np.asarray(x, dtype=np.float32)
    rel_bias = np.asarray(rel_bias, dtype=np.float32)
    scale = 1.0 / math.sqrt(HD)

    # xt[k, tb, dt, t] = x[tb*TB + t, dt*128 + k] -> one contiguous run per
    # partition per token-block DMA
    xt = (
        x.reshape(TOK, D)
        .T.reshape(DT, 128, NTB, TB)
        .transpose(1, 2, 0, 3)
    )
    xt = np.ascontiguousarray(xt).astype(ml_dtypes.bfloat16)

    # exp-bias tiles: ebt[i, hh, cls, j] = exp(rel_bias[(c0 - i + j) % PERIOD, h])
    ii = np.arange(128)[:, None]
    jj = np.arange(SQ)[None, :]
    cls_idx = np.empty((NCLS, 128, SQ), dtype=np.int64)
    for cls in range(NCLS):
        c0 = 128 * (cls - (NKT - 1))
        cls_idx[cls] = (c0 - ii + jj) % PERIOD

    in_maps = []
    for c in range(NCORES):
        sl = slice(c * CH, (c + 1) * CH)
        wq_c = (np.asarray(wq, np.float32)[:, sl] * scale).reshape(DT, 128, CH)
        wk_c = np.asarray(wk, np.float32)[:, sl].reshape(DT, 128, CH)
        wv_c = np.asarray(wv, np.float32)[:, sl].reshape(DT, 128, CH)
        eb = np.empty((128, 2, NCLS, SQ), dtype=ml_dtypes.bfloat16)
        for hh in range(2):
            h = 2 * c + hh
            eb[:, hh] = np.exp(rel_bias[cls_idx, h]).transpose(1, 0, 2)
        in_maps.append(
            {
                "xt": xt,
                "wq": np.ascontiguousarray(wq_c.transpose(1, 0, 2)).astype(ml_dtypes.bfloat16),
                "wk": np.ascontiguousarray(wk_c.transpose(1, 0, 2)).astype(ml_dtypes.bfloat16),
                "wv": np.ascontiguousarray(wv_c.transpose(1, 0, 2)).astype(ml_dtypes.bfloat16),
                "wo": np.ascontiguousarray(np.asarray(wo, np.float32)[sl, :]).astype(ml_dtypes.bfloat16),
                "bq": (np.asarray(bq, np.float32)[sl] * scale).reshape(CH, 1),
                "bk": np.asarray(bk, np.float32)[sl].reshape(CH, 1),
                "bv": np.asarray(bv, np.float32)[sl].reshape(CH, 1),
                "ebias": eb,
            }
        )
    return in_maps


def kernel(x, wq, bq, wk, bk, wv, bv, wo, bo, rel_bias, _trace=False):
    from concourse import bass_utils

    if "nc" not in _CACHE:
        _CACHE["nc"] = _build_nc()
    nc = _CACHE["nc"]

    in_maps = _prep_inputs(x, wq, bq, wk, bk, wv, bv, wo, bo, rel_bias)
    res = bass_utils.run_bass_kernel_spmd(
        nc, in_maps, core_ids=list(range(NCORES)), trace=_trace
    )
    _CACHE["last_result"] = res

    acc = np.zeros((TOK, D), dtype=np.float32)
    for r in res.results:
        acc += r["o_part"].astype(np.float32)
    acc += np.asarray(bo, np.float32)[None, :]
    return acc.reshape(B, S, D)


# revision 30
# speedup vs baseline: 1.1269x; 1.1269x over previous
"""Circular-relative-bias multi-head attention on 8 Trainium2 NeuronCores.

Sharding (Megatron MHA): 16 heads -> 2 heads per core. Each core computes
q/k/v projections for its 128 channels (2 heads x 64), full attention for
its heads over both batches, and a row-sharded output projection producing
a full-shape partial; the host sums the 8 partials and adds bo.

Layout strategy: the host pre-packs every input into the exact SBUF layout
the kernel wants, so all DMAs are linear:
  - xt      [128, 16, 8, 256]   x transposed, tiled per token-block so each
                                block is one contiguous run per partition
  - wq/wk/wv[128, 8, 128]       [k-part, d-tile, ch]         bf16 (wq,bq pre-scaled 1/8)
  - wo      [128, 1024]         [ch, d]                      bf16
  - bq/bk/bv[128, 1]            per-channel bias             f32
  - ebias   [128, 2, 28, 512]   exp(rel bias) tiles, by      bf16
                                [i, head, diag-class, j]

Attention works on transposed scores P^T [sk, sq] so softmax sums come free
from the attn@V matmul via a ones-column in the stationary operand:
  head0 lhsT = v_store[:, t, 0:65]  = [v0 | ones]      -> accA rows 0-63 data, 64 sums
  head1 lhsT = v_store[:, t, 32:160] = [.. | ones | v1] -> accB row 32 sums, rows 64-127 data

exp(s + b) = exp(s) * exp(b): the circular bias enters as a precomputed
elementwise bf16 multiplier; [128, 512] score tiles along the same
(512*qb - 128*kt) diagonal share one of 28 classes per head.

Engine schedule: ONE flat software pipeline over all 128 (b, qb, kt)
iterations -- QK is issued two slots ahead and PV lags one slot behind, both
crossing qb boundaries, so the per-slot steady state (exp on scalar ~1.0us,
bias-mul on vector/gpsimd, 2xQK + 2xPV on tensor) never drains at a qb edge.
Per-qb epilogue work (reciprocal straight from PSUM, gpsimd partition
broadcast, normalize muls, 8 output-projection matmuls + psum->sbuf casts +
HBM writeback) is sliced into per-slot deferred closures spread across the
following qb's 16 slots. PSUM: scores 2x2 banks, PV accum 3x1, outproj 1x1.

No max-subtraction in softmax: scores ~ N(0,1) + 0.02-bias, |s| < ~7 over
4M samples, exp stays well inside f32 range.
"""

import math
from collections import defaultdict

import numpy as np
import ml_dtypes

B = 2
S = 2048
D = 1024
H = 16
HD = 64
PERIOD = 4096
NCORES = 8
CH = 128          # channels per core = 2 heads * 64
TOK = B * S       # 4096
DT = D // 128     # 8 k-tiles for the d contraction
TB = 256          # token block for projections
NTB = TOK // TB   # 16
SQ = 512          # sq block in attention
NQB = S // SQ     # 4 per batch
SK = 128          # sk tile
NKT = S // SK     # 16 per batch
NCLS = NQB * 4 + NKT - 4  # 28 diagonal classes: 4*qb - kt in [-15, 12]
NIT = B * NQB * NKT       # 128 flat attention iterations

_CACHE = {}


def _build_nc():
    import contextlib

    import concourse.tile as tile
    from concourse import bacc, mybir
    from concourse.masks import make_identity

    f32 = mybir.dt.float32
    bf16 = mybir.dt.bfloat16

    nc = bacc.Bacc("TRN2")
    xt = nc.dram_tensor("xt", [128, NTB, DT, TB], bf16, kind="ExternalInput")
    wq = nc.dram_tensor("wq", [128, DT, CH], bf16, kind="ExternalInput")
    wk = nc.dram_tensor("wk", [128, DT, CH], bf16, kind="ExternalInput")
    wv = nc.dram_tensor("wv", [128, DT, CH], bf16, kind="ExternalInput")
    wo = nc.dram_tensor("wo", [CH, D], bf16, kind="ExternalInput")
    bq = nc.dram_tensor("bq", [CH, 1], f32, kind="ExternalInput")
    bk = nc.dram_tensor("bk", [CH, 1], f32, kind="ExternalInput")
    bv = nc.dram_tensor("bv", [CH, 1], f32, kind="ExternalInput")
    ebias = nc.dram_tensor("ebias", [128, 2, NCLS, SQ], bf16, kind="ExternalInput")
    o_part = nc.dram_tensor("o_part", [TOK, D], bf16, kind="ExternalOutput")

    with tile.TileContext(nc) as tc, contextlib.ExitStack() as ctx:
        singles = ctx.enter_context(tc.tile_pool(name="singles", bufs=1))
        xt_pool = ctx.enter_context(tc.tile_pool(name="xt", bufs=6))
        vt_pool = ctx.enter_context(tc.tile_pool(name="vt", bufs=2))
        e_pool = ctx.enter_context(tc.tile_pool(name="ep", bufs=3))
        p_pool = ctx.enter_context(tc.tile_pool(name="pp", bufs=4))
        nrm_pool = ctx.enter_context(tc.tile_pool(name="nrm", bufs=6))
        ao_pool = ctx.enter_context(tc.tile_pool(name="ao", bufs=2))
        ob_pool = ctx.enter_context(tc.tile_pool(name="ob", bufs=3))
        # PSUM budget (8 banks): scores 2x2 + PV accum 3x1 + outproj 1x1.
        sc_ps = ctx.enter_context(tc.tile_pool(name="scps", bufs=2, space="PSUM"))
        acc_ps = ctx.enter_context(tc.tile_pool(name="accps", bufs=3, space="PSUM"))
        op_ps = ctx.enter_context(tc.tile_pool(name="opps", bufs=1, space="PSUM"))

        ident = singles.tile([128, 128], bf16)
        make_identity(nc, ident)

        # q^T / k^T stores [ch, tok]; v_store [tok-part, tok-tile, 160]
        qT = singles.tile([CH, TOK], bf16, tag="qT")
        kT = singles.tile([CH, TOK], bf16, tag="kT")
        # v_store cols: [v0: 0..63 | ones: 64 | zeros: 65..95 | v1: 96..159]
        v_store = singles.tile([128, TOK // 128, 160], bf16, tag="vst")
        nc.vector.memset(v_store[:, :, 64:65], 1.0)
        nc.vector.memset(v_store[:, :, 65:96], 0.0)

        # ---- input DMAs: first xt tiles win early HBM bandwidth ----
        w_sb = {}
        b_sb = {}
        for name, w_h, b_h in (("q", wq, bq), ("k", wk, bk), ("v", wv, bv)):
            w_sb[name] = singles.tile([128, DT, CH], bf16, tag=f"w{name}", name=f"w{name}_sb")
            nc.sync.dma_start(out=w_sb[name], in_=w_h[:, :, :])
            b_sb[name] = singles.tile([CH, 1], f32, tag=f"b{name}", name=f"b{name}_sb")
            nc.sync.dma_start(out=b_sb[name], in_=b_h[:, :])

        # attention-phase weights, interleaved into the projection loop so
        # they stream behind the xt tiles without delaying them
        eb_sb = singles.tile([128, 2, NCLS, SQ], bf16, tag="eb")
        wo_sb = singles.tile([CH, D], bf16, tag="wo")

        # ---- projections ----
        for tb in range(NTB):
            ts = tb * TB
            xt_sb = xt_pool.tile([128, DT, TB], bf16, tag="xt")
            nc.sync.dma_start(out=xt_sb, in_=xt[:, tb])
            for name, store in (("q", qT), ("k", kT), ("v", None)):
                ps = sc_ps.tile([CH, TB], f32, tag="mm")
                for dt in range(DT):
                    nc.tensor.matmul(
                        ps,
                        w_sb[name][:, dt],
                        xt_sb[:, dt],
                        start=(dt == 0),
                        stop=(dt == DT - 1),
                    )
                if store is not None:
                    nc.scalar.add(store[:, ts : ts + TB], ps, b_sb[name])
                else:
                    vt_sb = vt_pool.tile([CH, TB], bf16, tag="vt")
                    nc.scalar.add(vt_sb, ps, b_sb["v"])
                    # transpose v^T -> v rows, split heads into v_store
                    for j in range(TB // 128):
                        t_idx = (ts + j * 128) // 128
                        vps = acc_ps.tile([128, 128], bf16, tag="acc")
                        nc.tensor.transpose(
                            vps, vt_sb[:, j * 128 : (j + 1) * 128], ident
                        )
                        nc.vector.tensor_copy(
                            v_store[:, t_idx, 0:64], vps[:, 0:64]
                        )
                        nc.vector.tensor_copy(
                            v_store[:, t_idx, 96:160], vps[:, 64:128]
                        )
            if tb == 5:
                nc.sync.dma_start(out=eb_sb[:, :, 0:8, :], in_=ebias[:, :, 0:8, :])
            elif tb == 10:
                nc.sync.dma_start(out=eb_sb[:, :, 8:16, :], in_=ebias[:, :, 8:16, :])
            elif tb == 13:
                nc.sync.dma_start(out=wo_sb, in_=wo[:, :])
            elif tb == 15:
                # classes 22-27 are first read in qb 2 (~35us into attention);
                # their DMA is deferred into the attention phase below
                nc.sync.dma_start(
                    out=eb_sb[:, :, 16:22, :], in_=ebias[:, :, 16:22, :]
                )

        # ---- attention: one flat software pipeline over 128 iterations ----
        def it_info(i):
            b, r = divmod(i, NQB * NKT)
            qb, kt = divmod(r, NKT)
            return b, qb, kt

        sc_tiles = {}
        acc = {}
        # pre_d runs at the top of a slot (amuls, psum->sbuf casts); post_d at
        # the bottom (outproj matmuls) so an outproj MM still waiting on the
        # normalize chain sits BEHIND that slot's QK/PV in the tensor FIFO
        # instead of head-of-line blocking them
        pre_d = defaultdict(list)
        post_d = defaultdict(list)

        def qk_flat(i):
            b, qb, kt = it_info(i)
            sc = sc_ps.tile([128, 2, SQ], f32, tag="mm", name=f"sc_{i}")
            sc_tiles[i] = sc
            base = b * S
            q0 = base + qb * SQ
            k0 = base + kt * SK
            for hh in (0, 1):
                nc.tensor.matmul(
                    sc[:, hh, :],
                    kT[hh * 64 : (hh + 1) * 64, k0 : k0 + SK],
                    qT[hh * 64 : (hh + 1) * 64, q0 : q0 + SQ],
                    start=True,
                    stop=True,
                )

        def emit_epilogue(slot, b, qb, accA, accB, last=False):
            """Normalize + outproj for finished qb, spread over next slots."""
            # reciprocal straight from the PSUM sum rows (head0: accA row 64,
            # head1: accB row 32), broadcast over the 64 head channels
            srow0 = nrm_pool.tile([1, SQ], f32, tag="srow")
            srow1 = nrm_pool.tile([1, SQ], f32, tag="srow")
            nc.vector.tensor_copy(srow0, accA[64:65])
            nc.vector.tensor_copy(srow1, accB[32:33])
            rr0 = nrm_pool.tile([1, SQ], f32, tag="rr")
            rr1 = nrm_pool.tile([1, SQ], f32, tag="rr")
            nc.vector.reciprocal_approx_fast(out=rr0, in_=srow0)
            nc.vector.reciprocal_approx_fast(out=rr1, in_=srow1)
            rb0 = nrm_pool.tile([64, SQ], f32, tag="rb")
            rb1 = nrm_pool.tile([64, SQ], f32, tag="rb")
            nc.gpsimd.partition_broadcast(rb0, rr0)
            nc.gpsimd.partition_broadcast(rb1, rr1)
            attnout = ao_pool.tile([128, SQ], bf16, tag="ao")

            def amuls():
                nc.vector.tensor_mul(attnout[0:64], accA[0:64], rb0)
                nc.vector.tensor_mul(attnout[64:128], accB[64:128], rb1)

            pre_d[slot + 1].append(amuls)

            base_q = b * S + qb * SQ
            if last:
                # drain tail: scores PSUM is free, so pipeline full-width
                # outproj tiles through sc_ps with casts on scalar+vector
                for ts in range(4):

                    def run2(ts=ts):
                        op = sc_ps.tile([128, 2, SQ], f32, tag="mm", name=f"opd_{ts}")
                        for half in range(2):
                            nc.tensor.matmul(
                                op[:, half, :],
                                attnout[:, ts * 128 : (ts + 1) * 128],
                                wo_sb[:, half * 512 : (half + 1) * 512],
                                start=True,
                                stop=True,
                            )

                        def cp(op=op, ts=ts):
                            ob = ob_pool.tile([128, 2, SQ], bf16, tag="ob2")
                            if ts % 2:
                                nc.scalar.copy(ob, op)
                            else:
                                nc.vector.tensor_copy(ob, op)
                            r0 = base_q + ts * 128
                            nc.sync.dma_start(out=o_part[r0 : r0 + 128, :], in_=ob)

                        pre_d[slot + 3 + ts].append(cp)

                    post_d[slot + 2 + ts].append(run2)
                return
            for k in range(8):
                ts, half = divmod(k, 2)

                def run_mm(ts=ts, half=half):
                    op = op_ps.tile([128, SQ], f32, tag="op")
                    nc.tensor.matmul(
                        op,
                        attnout[:, ts * 128 : (ts + 1) * 128],
                        wo_sb[:, half * 512 : (half + 1) * 512],
                        start=True,
                        stop=True,
                    )

                    def run_cp(op=op, ts=ts, half=half, k=k):
                        ob = ob_pool.tile([128, SQ], bf16, tag="ob")
                        # balance psum->sbuf casts: 2 of 8 ride the scalar
                        # queue (vector is the tightest engine in steady state)
                        if k in (1, 5):
                            nc.scalar.copy(ob, op)
                        else:
                            nc.vector.tensor_copy(ob, op)
                        r0 = base_q + ts * 128
                        nc.sync.dma_start(
                            out=o_part[r0 : r0 + 128, half * 512 : (half + 1) * 512],
                            in_=ob,
                        )

                    pre_d[slot + 5 + 2 * k].append(run_cp)

                post_d[slot + 4 + 2 * k].append(run_mm)

        def emit_pv(j):
            b, qb, kt = it_info(j)
            if kt == 0:
                acc[(b, qb)] = (
                    acc_ps.tile([128, SQ], f32, tag="acc", name=f"accA_{b}_{qb}"),
                    acc_ps.tile([128, SQ], f32, tag="acc", name=f"accB_{b}_{qb}"),
                )
            accA, accB = acc[(b, qb)]
            t_idx = (b * S + kt * SK) // 128
            p_sb = p_tiles.pop(j)
            for hh, a in ((0, accA), (1, accB)):
                lo, width = (0, 65) if hh == 0 else (32, 128)
                nc.tensor.matmul(
                    a[0:width, :],
                    v_store[:, t_idx, lo : lo + width],
                    p_sb[:, hh, :],
                    start=(kt == 0),
                    stop=(kt == NKT - 1),
                )
            return kt == NKT - 1

        # bias-muls on these kt go to gpsimd (~3x slower than DVE), so their
        # PV is pushed one extra slot to keep the tensor FIFO from stalling
        GP_KTS = set()  # gpsimd elementwise offload measured slower on hw
        p_tiles = {}
        pv_due = defaultdict(list)
        for i in range(NIT + 24):
            for fn in pre_d.pop(i, []):
                fn()
            if i == 0:
                qk_flat(0)
                qk_flat(1)
            if i == 2:
                nc.sync.dma_start(
                    out=eb_sb[:, :, 22:28, :], in_=ebias[:, :, 22:28, :]
                )
            if i < NIT:
                b, qb, kt = it_info(i)
                cls = 4 * qb - kt + (NKT - 1)
                e_sb = e_pool.tile([128, 2, SQ], bf16, tag="e")
                nc.scalar.activation(
                    out=e_sb, in_=sc_tiles.pop(i),
                    func=mybir.ActivationFunctionType.Exp,
                )
                p_sb = p_pool.tile([128, 2, SQ], bf16, tag="p")
                eng = nc.gpsimd if kt in GP_KTS else nc.vector
                eng.tensor_mul(p_sb, e_sb, eb_sb[:, :, cls, :])
                p_tiles[i] = p_sb
                pv_due[i + (2 if kt in GP_KTS else 1)].append(i)
                if i + 2 < NIT:
                    qk_flat(i + 2)
            for j in sorted(pv_due.pop(i, [])):
                if emit_pv(j):
                    jb, jqb, _ = it_info(j)
                    a, bacc_t = acc[(jb, jqb)]
                    emit_epilogue(i, jb, jqb, a, bacc_t, last=(j == NIT - 1))
            for fn in post_d.pop(i, []):
                fn()
        assert not pre_d and not post_d and not p_tiles and not sc_tiles
    nc.compile()
    return nc


def _prep_inputs(x, wq, bq, wk, bk, wv, bv, wo, bo, rel_bias):
    """Host-side pack into per-core in_maps (all linear-DMA layouts)."""
    x = np.asarray(x, dtype=np.float32)
    rel_bias = np.asarray(rel_bias, dtype=np.float32)
    scale = 1.0 / math.sqrt(HD)

    # xt[k, tb, dt, t] = x[tb*TB + t, dt*128 + k] -> one contiguous run per
    # partition per token-block DMA
    xt = (
        x.reshape(TOK, D)
        .T.reshape(DT, 128, NTB, TB)
        .transpose(1, 2, 0, 3)
    )
    xt = np.ascontiguousarray(xt).astype(ml_dtypes.bfloat16)

    # exp-bias tiles: ebt[i, hh, cls, j] = exp(rel_bias[(c0 - i + j) % PERIOD, h])
    ii = np.arange(128)[:, None]
    jj = np.arange(SQ)[None, :]
    cls_idx = np.empty((NCLS, 128, SQ), dtype=np.int64)
    for cls in range(NCLS):
        c0 = 128 * (cls - (NKT - 1))
        cls_idx[cls] = (c0 - ii + jj) % PERIOD

    in_maps = []
    for c in range(NCORES):
        sl = slice(c * CH, (c + 1) * CH)
        wq_c = (np.asarray(wq, np.float32)[:, sl] * scale).reshape(DT, 128, CH)
        wk_c = np.asarray(wk, np.float32)[:, sl].reshape(DT, 128, CH)
        wv_c = np.asarray(wv, np.float32)[:, sl].reshape(DT, 128, CH)
        eb = np.empty((128, 2, NCLS, SQ), dtype=ml_dtypes.bfloat16)
        for hh in range(2):
            h = 2 * c + hh
            eb[:, hh] = np.exp(rel_bias[cls_idx, h]).transpose(1, 0, 2)
        in_maps.append(
            {
                "xt": xt,
                "wq": np.ascontiguousarray(wq_c.transpose(1, 0, 2)).astype(ml_dtypes.bfloat16),
                "wk": np.ascontiguousarray(wk_c.transpose(1, 0, 2)).astype(ml_dtypes.bfloat16),
                "wv": np.ascontiguousarray(wv_c.transpose(1, 0, 2)).astype(ml_dtypes.bfloat16),
                "wo": np.ascontiguousarray(np.asarray(wo, np.float32)[sl, :]).astype(ml_dtypes.bfloat16),
                "bq": (np.asarray(bq, np.float32)[sl] * scale).reshape(CH, 1),
                "bk": np.asarray(bk, np.float32)[sl].reshape(CH, 1),
                "bv": np.asarray(bv, np.float32)[sl].reshape(CH, 1),
                "ebias": eb,
            }
        )
    return in_maps


def kernel(x, wq, bq, wk, bk, wv, bv, wo, bo, rel_bias, _trace=False):
    from concourse import bass_utils

    if "nc" not in _CACHE:
        _CACHE["nc"] = _build_nc()
    nc = _CACHE["nc"]

    in_maps = _prep_inputs(x, wq, bq, wk, bk, wv, bv, wo, bo, rel_bias)
    res = bass_utils.run_bass_kernel_spmd(
        nc, in_maps, core_ids=list(range(NCORES)), trace=_trace
    )
    _CACHE["last_result"] = res

    acc = np.zeros((TOK, D), dtype=np.float32)
    for r in res.results:
        acc += r["o_part"].astype(np.float32)
    acc += np.asarray(bo, np.float32)[None, :]
    return acc.reshape(B, S, D)
